# revision 12
# baseline (speedup 1.0000x reference)
"""Trainium2 Bass kernel for a 2-layer edge-gated GCN (DiffGNNPlacement).

Math (reference, per layer):
    ew   = 0.5 + sigmoid(edge_logits)                  # [E]
    deg  = segsum(ew -> col) + 1                       # [N]
    dis  = deg^-1/2
    norm = dis[row] * ew * dis[col]                    # [E]
    out  = segsum(norm * (h@W)[row] -> col) + (h@W)*dis^2 + b

Aggregation commutes with the linear transform, so each phase aggregates the
raw features and applies the dense transform afterwards (phase A), or
aggregates pre-transformed features (phase B aggregates h1@W2, 32ch).

Device algorithm (per core, nodes sharded 12500/core, 2 programs):
  - per-edge rows gathered via SWDGE dma_gather with sub-256B elem sizes
    (128B in A, 64B in B); the feature tables are packed 2 resp. 4 nodes per
    256B stride cell, and the 4 streams per core (one per SWDGE queue) are
    keyed by (chunk, sub-offset) so the int16 gather indices address packed
    cells while the dma base encodes the sub-cell byte offset.
  - all gather indices are SBUF-resident from program start (one DMA), so
    the Q7 descriptor loops never stall on mid-stream index loads.
  - per tile: 128 gathered rows x S[128, 32] one-hot-times-norm bf16 matmul
    accumulated into a [CH, 512] PSUM window.
  - per window (transposed pipeline): z = psum + sxT; phase A: h =
    relu(W1^T z + b1), hp = W2^T h (32ch), transposed back via identity
    matmuls and written as the packed phase-B table; phase B: relu(z + b2),
    classifier head -> outT.
Host does structure-only planning (norms, sorting, one-hot S, index packing)
plus the inter-phase table relay, like the two-phase baseline.
"""

import os
import sys
import numpy as np
from contextlib import ExitStack

for _p in ("/opt/trn_rl_repo", "/root/.axon_site/_ro/trn_rl_repo"):
    if os.path.isdir(_p) and _p not in sys.path:
        sys.path.insert(0, _p)

import ml_dtypes

BF16 = np.dtype(ml_dtypes.bfloat16)


# ----------------------------------------------------------------- config ---
class Cfg:
    def __init__(self):
        self.N = 100000
        self.E = 1600000
        self.C = 64           # feature channels (phase A)
        self.H2 = 32          # phase B channels
        self.P = 8
        self.NLOC = self.N // self.P          # 12500
        self.W = 48           # S tile width (99.7% tile fill at 4 edges/col)
        self.WIN = 512        # PSUM window
        self.TCH = 32         # tiles per gather chunk
        self.NWIN = (self.NLOC + self.WIN - 1) // self.WIN
        # phase A: x packed 2 nodes / 256B cell -> 50000 cells, 2 chunks of
        # 25000 cells (int16), sub in {0,1}: queue = chunk*2 + sub
        self.A_CELLS = self.N // 2            # 50000
        self.A_CHUNK = 25000
        # phase B: h1p packed 4 rows / 256B group; per-shard padded rows
        self.TPP = 98                          # table rows per partition
        self.NLOC_PAD = 128 * self.TPP         # 12544
        self.B_ROWS = self.P * self.NLOC_PAD   # 100352
        self.B_CELLS = self.B_ROWS // 4        # 25088 (fits int16)


FULL = Cfg()


# ------------------------------------------------------- raw gather (sub-256B)
def dma_gather_raw(gp, out_ap, in_ap, idxs_ap, num_idxs, elem_size,
                   elem_step, queue_num):
    """nc.gpsimd.dma_gather minus the elem_size_bytes%256 assert (the
    non-transpose ucode/decode path supports arbitrary elem sizes; only the
    table stride must be a multiple of 256B)."""
    import concourse.mybir as mybir
    from concourse.bass import MemorySpace
    from concourse import ap_utils

    gp._assert_queue_num(queue_num)
    assert idxs_ap.dtype == mybir.dt.int16
    assert in_ap.space == MemorySpace.DRAM
    assert in_ap.dtype == out_ap.dtype
    assert idxs_ap.space == MemorySpace.SBUF
    assert out_ap.space == MemorySpace.SBUF
    assert ap_utils.ap_is_contiguous(in_ap.ap[1:])
    assert ap_utils.ap_is_contiguous(out_ap.ap[1:])
    assert ap_utils.ap_is_contiguous(idxs_ap.ap[1:])
    assert in_ap.ap[-1][1] == out_ap.ap[-1][1] == elem_size
    assert out_ap.ap[0][1] * out_ap.ap[1][1] == num_idxs  # num_idxs % 128 == 0
    assert in_ap.ap[0][0] == elem_step
    stride_bytes = elem_step * mybir.dt.size(in_ap.dtype)
    stride_bytes_256, rem = divmod(stride_bytes, 256)
    assert rem == 0 and 0 < stride_bytes_256 < 256

    _in_ap = gp.lower_ap_dma(in_ap, for_custom_bir_dma=True)
    _idxs_ap = gp.lower_ap(idxs_ap)
    _out_ap = gp.lower_ap(out_ap)
    return gp.add_instruction(
        mybir.InstDMAGatherAnt(
            name=gp.bass.get_next_instruction_name(),
            ins=[*_in_ap, _idxs_ap,
                 gp.lower_val_access(gp.to_reg(num_idxs))],
            outs=[_out_ap],
            transpose=False,
            num_idxs=num_idxs,
            elem_size=elem_size,
            stride_bytes_256=stride_bytes_256,
            gen_mode=0,
            single_packet=False,
            queue_num=queue_num,
            sbuf_tokens_per_rank=0,
            sbuf_free_dim_per_rank=0,
            sbuf_free_dim_pad_per_rank=0,
            sbuf_byte_offset=0,
        )
    )


# --------------------------------------------------------- host preprocess ---
def _sigmoid(x):
    return 0.5 * (np.tanh(0.5 * x) + 1.0)


def _cell_of_local(sl, cfg):
    """phase-B packed table row of local node sl (partition-major layout)."""
    return (sl % 128) * cfg.TPP + sl // 128


def _plan_stream(units, cols, vals, cfg):
    """Tile one sorted-by-col edge stream: 128-slot tiles, <=W col span.
    units: int16 gather indices (packed-cell units). Returns packed idx
    (16-part wrapped) + S arrays + window placement."""
    m = len(cols)
    starts, c0s = [], []
    i = 0
    while i < m:
        c0 = int(cols[i])
        jmax = min(i + 128, m)
        j = i + int(np.searchsorted(cols[i:jmax], c0 + cfg.W, side="left"))
        starts.append(i)
        c0s.append(c0)
        i = j
    T = len(c0s)
    starts_a = np.array(starts + [m], dtype=np.int64)
    c0s = np.array(c0s, dtype=np.int32)

    tile_of = np.repeat(np.arange(T), np.diff(starts_a))
    slot = np.arange(m) - starts_a[tile_of]
    idx16 = np.zeros((T, 128), np.int16)
    idx16[tile_of, slot] = units
    S = np.zeros((T, 128, cfg.W), np.float32)
    S[tile_of, slot, cols - c0s[tile_of]] = vals

    TCH = cfg.TCH
    nch = max(1, (T + TCH - 1) // TCH)
    Tp = nch * TCH
    flat = np.zeros(Tp * 128, np.int16)
    flat[: T * 128] = idx16.reshape(-1)
    # wrap: slot k*16+j -> [j, k]
    wrapped = flat.reshape(nch, TCH * 128 // 16, 16).transpose(0, 2, 1)
    idx_w = np.ascontiguousarray(wrapped)                 # [nch,16,TCH*8]
    Sp = np.zeros((Tp, 128, cfg.W), np.float32)
    Sp[:T] = S
    S_pk = np.ascontiguousarray(
        Sp.reshape(nch, TCH, 128, cfg.W).transpose(0, 2, 1, 3)).astype(BF16)

    win = c0s // cfg.WIN
    off = c0s - win * cfg.WIN
    return dict(T=T, nch=nch, idx=idx_w, S=S_pk, win=win, off=off)


def preprocess(edge_index, edge_logits, cfg=FULL):
    """Norms + per-device tile plans for both phases (pure numpy)."""
    N, NLOC = cfg.N, cfg.NLOC
    row = np.asarray(edge_index[0], dtype=np.int64)
    col = np.asarray(edge_index[1], dtype=np.int64)
    ew = (0.5 + _sigmoid(np.asarray(edge_logits, dtype=np.float32))).astype(np.float32)
    deg = np.bincount(col, weights=ew.astype(np.float64), minlength=N).astype(np.float32) + 1.0
    dis = deg ** -0.5
    norm = (dis[row] * ew * dis[col]).astype(np.float32)

    dev = col // NLOC
    lcol = (col - dev * NLOC).astype(np.int32)

    # phase A stream key: queue = (row//50000)*2 + row%2, unit = row//2 - chunk*25000
    qa = (row // (cfg.A_CHUNK * 2)) * 2 + (row % 2)
    ua = (row // 2) - (row // (cfg.A_CHUNK * 2)) * cfg.A_CHUNK
    # phase B: packed global row -> queue rB%4, unit rB//4
    ds = row // NLOC
    sl = row % NLOC
    rB = ds * cfg.NLOC_PAD + _cell_of_local(sl, cfg)
    qb = rB % 4
    ub = rB // 4
    assert ub.max() < 32768 and ua.max() < 32768

    plans = {"A": [], "B": []}
    for phase, q, u in (("A", qa, ua), ("B", qb, ub)):
        order = np.lexsort((lcol, q, dev))
        so_u, so_c, so_v = u[order].astype(np.int16), lcol[order], norm[order]
        so_d, so_q = dev[order], q[order]
        key = so_d * 4 + so_q
        bounds = np.searchsorted(key, np.arange(cfg.P * 4 + 1))
        for d in range(cfg.P):
            qplans = []
            for g in range(4):
                a, b = bounds[d * 4 + g], bounds[d * 4 + g + 1]
                qplans.append(_plan_stream(so_u[a:b], so_c[a:b], so_v[a:b], cfg))
            plans[phase].append(qplans)
    return plans, dis


def pack_idx(qplans, cfg):
    """Assemble the resident idx tile [128, NCHMAX*TCH*8] int16: queue q's
    wrapped indices replicated to partitions [32q,32q+16) and [32q+16,32q+32)."""
    nchmax = max(p["nch"] for p in qplans)
    width = nchmax * cfg.TCH * 8
    out = np.zeros((128, width), np.int16)
    for q, p in enumerate(qplans):
        flat = p["idx"].transpose(1, 0, 2).reshape(16, -1)  # [16, nch*TCH*8]
        out[32 * q:32 * q + 16, : flat.shape[1]] = flat
        out[32 * q + 16:32 * q + 32, : flat.shape[1]] = flat
    return out, nchmax


# ---------------------------------------------------------- program builder ---
def build_program(qplans, phase, cfg=FULL, name="gnn"):
    import concourse.bass as bass
    import concourse.mybir as mybir
    from concourse import bacc
    from concourse.tile import TileContext

    f32, bf16, i16 = mybir.dt.float32, mybir.dt.bfloat16, mybir.dt.int16
    W, WIN, TCH, NLOC = cfg.W, cfg.WIN, cfg.TCH, cfg.NLOC
    CH = cfg.C if phase == "A" else cfg.H2   # gathered/agg channels

    nc = bacc.Bacc("TRN2", enable_partition_id=False,
                   target_bir_lowering=False, name=name,
                   num_swdge_queues=4)

    if phase == "A":
        table = nc.dram_tensor("table", [cfg.A_CELLS, 128], bf16, kind="ExternalInput")
    else:
        table = nc.dram_tensor("table", [cfg.B_CELLS, 128], bf16, kind="ExternalInput")
    sxT_dr = nc.dram_tensor("sxT", [CH, NLOC], f32, kind="ExternalInput")
    _, nchmax = pack_idx(qplans, cfg)
    idx_dr = nc.dram_tensor("idxall", [128, nchmax * TCH * 8], i16, kind="ExternalInput")
    S_dr = []
    for q in range(4):
        p = qplans[q]
        S_dr.append(nc.dram_tensor(f"S{q}", list(p["S"].shape), bf16,
                                   kind="ExternalInput"))
    if phase == "A":
        W1_dr = nc.dram_tensor("W1", [64, 64], f32, kind="ExternalInput")
        b1_dr = nc.dram_tensor("b1c", [64, 1], f32, kind="ExternalInput")
        W2_dr = nc.dram_tensor("W2p", [64, 32], f32, kind="ExternalInput")
        id_dr = nc.dram_tensor("ident", [32, 32], bf16, kind="ExternalInput")
        h_out = nc.dram_tensor("h_out", [cfg.NLOC_PAD, 32], bf16, kind="ExternalOutput")
        dst3 = h_out.rearrange("(p t) c -> p t c", p=128)
    else:
        b2_dr = nc.dram_tensor("b2c", [32, 1], f32, kind="ExternalInput")
        lw_dr = nc.dram_tensor("lw", [32, 1], f32, kind="ExternalInput")
        lb_dr = nc.dram_tensor("lb", [1, 1], f32, kind="ExternalInput")
        outT = nc.dram_tensor("outT", [2, NLOC], f32, kind="ExternalOutput")

    # per-window tile lists: (q, t, off, s_lo, weff); straddling tiles split
    win_tiles = [[] for _ in range(cfg.NWIN)]
    for q in range(4):
        p = qplans[q]
        for t in range(p["T"]):
            w = int(p["win"][t])
            off = int(p["off"][t])
            wlen = min(WIN, NLOC - w * WIN)
            w1 = min(W, WIN - off)
            win_tiles[w].append((q, t, off, 0, min(w1, wlen - off)))
            if W > w1 and w + 1 < cfg.NWIN:
                wlen2 = min(WIN, NLOC - (w + 1) * WIN)
                win_tiles[w + 1].append((q, t, 0, w1, min(W - w1, wlen2)))

    with TileContext(nc) as tc, ExitStack() as ex:
        cpool = ex.enter_context(tc.tile_pool(name="consts", bufs=1))
        zpool = ex.enter_context(tc.tile_pool(name="z", bufs=3))
        gpools = [ex.enter_context(tc.tile_pool(name=f"gat{q}", bufs=3)) for q in range(4)]
        spools = [ex.enter_context(tc.tile_pool(name=f"s{q}", bufs=4)) for q in range(4)]
        ppool = ex.enter_context(tc.tile_pool(name="psagg", bufs=3, space="PSUM"))
        if phase == "A":
            pstpool = ex.enter_context(tc.tile_pool(name="psd", bufs=2, space="PSUM"))
            hppool = ex.enter_context(tc.tile_pool(name="psh", bufs=2, space="PSUM"))
            tppool = ex.enter_context(tc.tile_pool(name="pst", bufs=1, space="PSUM"))
            htpool = ex.enter_context(tc.tile_pool(name="ht", bufs=2))
            hptpool = ex.enter_context(tc.tile_pool(name="hpt", bufs=2))
            hspool = ex.enter_context(tc.tile_pool(name="hs", bufs=3))
        else:
            plpool = ex.enter_context(tc.tile_pool(name="psl", bufs=2, space="PSUM"))
            htpool = ex.enter_context(tc.tile_pool(name="ht", bufs=2))
            opool = ex.enter_context(tc.tile_pool(name="ot", bufs=3))

        # resident gather indices, loaded in two stages so chunk-0/1 gathers
        # start without waiting for the full index image
        head = min(2, nchmax) * TCH * 8
        idx_sb0 = cpool.tile([128, head], i16)
        nc.sync.dma_start(out=idx_sb0[:, :], in_=idx_dr[:, :head])
        idx_sb1 = None
        if nchmax > 2:
            idx_sb1 = cpool.tile([128, (nchmax - 2) * TCH * 8], i16)

        def idx_slice(ch, ntl):
            lo, hi = ch * TCH * 8, (ch * TCH + ntl) * 8
            if hi <= head:
                return idx_sb0[:, lo:hi]
            return idx_sb1[:, lo - head:hi - head]

        # last two chunks stay live per queue: a window-straddling tile's
        # continuation may be processed after the next chunk was entered
        # (pool bufs=4 keeps both chunks' tiles valid)
        cur = [{} for _ in range(4)]

        def table_slice(q):
            if phase == "A":
                g, s = q // 2, q % 2
                return table[g * cfg.A_CHUNK:(g + 1) * cfg.A_CHUNK,
                             s * 64:(s + 1) * 64]
            return table[:, (q % 4) * 32:(q % 4) * 32 + 32]

        def ensure_chunk(q, ch):
            st = cur[q]
            if ch in st:
                return st[ch]
            p = qplans[q]
            ntl = min(TCH, p["T"] - ch * TCH)
            nid = ntl * 128
            gb = gpools[q].tile([128, TCH, CH], bf16, tag="g")
            dma_gather_raw(
                nc.gpsimd, gb[:, :ntl, :], table_slice(q),
                idx_slice(ch, ntl),
                nid, CH, 128, q)
            sb = spools[q].tile([128, TCH, W], bf16, tag="s")
            nc.scalar.dma_start(out=sb[:, :ntl, :], in_=S_dr[q][ch, :, :ntl, :])
            st[ch] = dict(gb=gb, sb=sb)
            for old in [k for k in st if k < ch - 1]:
                del st[old]
            return st[ch]

        for q in range(4):
            ensure_chunk(q, 0)
        if idx_sb1 is not None:
            nc.sync.dma_start(out=idx_sb1[:, :], in_=idx_dr[:, head:])
        # whole self-loop term resident: window chains never wait on a DMA
        sxT_sb = cpool.tile([CH, NLOC], f32)
        nc.sync.dma_start(out=sxT_sb[:, :], in_=sxT_dr[:, :])

        # ---- constants
        zrow = cpool.tile([1, WIN], bf16)
        nc.vector.memset(zrow[:, :], 0.0)
        if phase == "A":
            W1_sb = cpool.tile([64, 64], f32)
            nc.sync.dma_start(out=W1_sb[:, :], in_=W1_dr[:, :])
            b1_sb = cpool.tile([64, 1], f32)
            nc.sync.dma_start(out=b1_sb[:, :], in_=b1_dr[:, :])
            W2_sb = cpool.tile([64, 32], f32)
            nc.sync.dma_start(out=W2_sb[:, :], in_=W2_dr[:, :])
            id_sb = cpool.tile([32, 32], bf16)
            nc.sync.dma_start(out=id_sb[:, :], in_=id_dr[:, :])
        else:
            b2_sb = cpool.tile([32, 1], f32)
            nc.sync.dma_start(out=b2_sb[:, :], in_=b2_dr[:, :])
            lw_sb = cpool.tile([32, 1], f32)
            nc.sync.dma_start(out=lw_sb[:, :], in_=lw_dr[:, :])
            lb_sb = cpool.tile([1, 1], f32)
            nc.sync.dma_start(out=lb_sb[:, :], in_=lb_dr[:, :])
            nlb = cpool.tile([1, 1], f32)
            nc.scalar.mul(nlb[:, :], lb_sb[:, :], -1.0)

        for w in range(cfg.NWIN):
            wlen = min(WIN, NLOC - w * WIN)
            ps = ppool.tile([CH, WIN], f32)
            nc.tensor.matmul(ps[:, :wlen], lhsT=zrow[:, :CH], rhs=zrow[:, :wlen],
                             start=True, stop=False)
            for q, t, off, s_lo, weff in win_tiles[w]:
                st = ensure_chunk(q, t // TCH)
                tp = t % TCH
                nc.tensor.matmul(
                    ps[:, off:off + weff],
                    lhsT=st["gb"][:, tp, :CH],
                    rhs=st["sb"][:, tp, s_lo:s_lo + weff],
                    start=False, stop=False,
                    skip_group_check=True,
                )
            nc.tensor.matmul(ps[:, :wlen], lhsT=zrow[:, :CH], rhs=zrow[:, :wlen],
                             start=False, stop=True)
            zw = zpool.tile([CH, WIN], f32, tag="z")
            nc.vector.tensor_tensor(out=zw[:, :wlen], in0=ps[:, :wlen],
                                    in1=sxT_sb[:, w * WIN:w * WIN + wlen],
                                    op=mybir.AluOpType.add)

            if phase == "A":
                pst = pstpool.tile([64, WIN], f32)
                nc.tensor.matmul(pst[:, :wlen], lhsT=W1_sb[:, :],
                                 rhs=zw[:, :wlen], start=True, stop=True)
                ht = htpool.tile([64, WIN], f32, tag="ht")
                nc.scalar.activation(ht[:, :wlen], pst[:, :wlen],
                                     mybir.ActivationFunctionType.Relu,
                                     bias=b1_sb[:, :])
                hp = hppool.tile([32, WIN], f32)
                nc.tensor.matmul(hp[:, :wlen], lhsT=W2_sb[:, :],
                                 rhs=ht[:, :wlen], start=True, stop=True)
                hpT = hptpool.tile([32, WIN], bf16, tag="hpT")
                nc.vector.tensor_copy(hpT[:, :wlen], hp[:, :wlen])
                nck = (wlen + 127) // 128
                hs = hspool.tile([128, 4, 32], bf16, tag="hs")
                for kk in range(nck):
                    mrow = min(128, wlen - kk * 128)
                    tpp = tppool.tile([128, 32], f32)
                    nc.tensor.matmul(tpp[:mrow, :],
                                     lhsT=hpT[:, kk * 128:kk * 128 + mrow],
                                     rhs=id_sb[:, :], start=True, stop=True)
                    nc.vector.tensor_copy(hs[:mrow, kk, :], tpp[:mrow, :])
                nc.sync.dma_start(out=dst3[:, w * 4:w * 4 + nck, :],
                                  in_=hs[:, :nck, :])
            else:
                ht2 = htpool.tile([32, WIN], f32, tag="ht2")
                nc.scalar.activation(ht2[:, :wlen], zw[:, :wlen],
                                     mybir.ActivationFunctionType.Relu,
                                     bias=b2_sb[:, :])
                psl = plpool.tile([1, WIN], f32)
                nc.tensor.matmul(psl[:, :wlen], lhsT=lw_sb[:, :],
                                 rhs=ht2[:, :wlen], start=True, stop=True)
                otn = opool.tile([1, WIN], f32, tag="otn")
                otp = opool.tile([1, WIN], f32, tag="otp")
                nc.scalar.activation(otn[:, :wlen], psl[:, :wlen],
                                     mybir.ActivationFunctionType.Identity,
                                     bias=nlb[:, :], scale=-1.0)
                nc.scalar.activation(otp[:, :wlen], psl[:, :wlen],
                                     mybir.ActivationFunctionType.Identity,
                                     bias=lb_sb[:, :], scale=1.0)
                nc.sync.dma_start(out=outT[0:1, w * WIN:w * WIN + wlen],
                                  in_=otn[:, :wlen])
                nc.sync.dma_start(out=outT[1:2, w * WIN:w * WIN + wlen],
                                  in_=otp[:, :wlen])

    nc.compile()
    return nc


# ------------------------------------------------------------------ runner ---
def make_runner(nc, device):
    """Single-core jit runner pinned to one device, reusable across calls."""
    import jax
    import concourse.mybir as mybir
    from concourse import bass2jax

    bass2jax.install_neuronx_cc_hook()

    in_names, out_names, out_avals, zero_shapes = [], [], [], []
    for alloc in nc.m.functions[0].allocations:
        if not isinstance(alloc, mybir.MemoryLocationSet):
            continue
        nm = alloc.memorylocations[0].name
        if alloc.kind == "ExternalInput":
            in_names.append(nm)
        elif alloc.kind == "ExternalOutput":
            shape = tuple(alloc.tensor_shape)
            dtype = mybir.dt.np(alloc.dtype)
            out_names.append(nm)
            out_avals.append(jax.core.ShapedArray(shape, dtype))
            zero_shapes.append((shape, dtype))
    n_params = len(in_names)
    all_in_names = in_names + out_names
    donate = tuple(range(n_params, n_params + len(out_names)))

    def _body(*args):
        outs = bass2jax._bass_exec_p.bind(
            *args,
            out_avals=tuple(out_avals),
            in_names=tuple(all_in_names),
            out_names=tuple(out_names),
            lowering_input_output_aliases=(),
            sim_require_finite=True,
            sim_require_nnan=True,
            nc=nc,
        )
        return tuple(outs)

    jitted = jax.jit(_body, donate_argnums=donate, keep_unused=True)

    def run(in_map):
        args = [jax.device_put(np.asarray(in_map[nm]), device) for nm in in_names]
        zeros = [jax.device_put(np.zeros(s, d), device) for s, d in zero_shapes]
        outs = jitted(*args, *zeros)
        return {nm: outs[i] for i, nm in enumerate(out_names)}

    return run


# ---------------------------------------------------------------- kernel() ---
_CACHE = {}


def _get_runners(plans, cfg):
    import jax
    from concurrent.futures import ThreadPoolExecutor
    key = "runners"
    if key in _CACHE:
        return _CACHE[key]
    devices = jax.devices()[:cfg.P]

    def build_pair(d):
        ncA = build_program(plans["A"][d], "A", cfg, name=f"gnnA_d{d}")
        ncB = build_program(plans["B"][d], "B", cfg, name=f"gnnB_d{d}")
        return (make_runner(ncA, devices[d]), make_runner(ncB, devices[d]))

    with ThreadPoolExecutor(4) as exe:
        runners = list(exe.map(build_pair, range(cfg.P)))
    _CACHE[key] = runners
    return runners


def run_two_phase(inputs, cfg=FULL):
    from concurrent.futures import ThreadPoolExecutor

    x = np.asarray(inputs["x"], np.float32)
    W1 = np.asarray(inputs["W1"], np.float32)
    b1 = np.asarray(inputs["b1"], np.float32)
    W2 = np.asarray(inputs["W2"], np.float32)
    b2 = np.asarray(inputs["b2"], np.float32)
    lin_w = np.asarray(inputs["lin_w"], np.float32)
    lin_b = np.asarray(inputs["lin_b"], np.float32)

    plans, dis = preprocess(inputs["edge_index"], inputs["edge_logits"], cfg)
    dis2 = (dis * dis).astype(np.float32)
    runners = _get_runners(plans, cfg)

    ident = np.eye(32, dtype=np.float32).astype(BF16)
    x_pack = np.ascontiguousarray(x.astype(BF16).reshape(cfg.A_CELLS, 128))

    def inputs_A(d):
        sh = slice(d * cfg.NLOC, (d + 1) * cfg.NLOC)
        sxT = np.ascontiguousarray((x[sh] * dis2[sh, None]).T)
        idxall, _ = pack_idx(plans["A"][d], cfg)
        m = dict(table=x_pack, sxT=sxT, idxall=idxall,
                 W1=W1, b1c=b1.reshape(64, 1), W2p=W2, ident=ident)
        for q in range(4):
            m[f"S{q}"] = plans["A"][d][q]["S"]
        return m

    with ThreadPoolExecutor(cfg.P) as exe:
        resA = list(exe.map(lambda d: runners[d][0](inputs_A(d)), range(cfg.P)))
    # assemble phase-B table: concat per-shard packed h1p rows
    h_shards = [np.asarray(r["h_out"]) for r in resA]       # [12544, 32] bf16
    tableB = np.ascontiguousarray(
        np.concatenate(h_shards, axis=0).reshape(cfg.B_CELLS, 128))

    def inputs_B(d):
        sh = slice(d * cfg.NLOC, (d + 1) * cfg.NLOC)
        # local h1p in node order: cell(p,t) = p*TPP+t holds node t*128+p
        hp_loc = h_shards[d].astype(np.float32).reshape(128, cfg.TPP, 32)
        hp_loc = hp_loc.transpose(1, 0, 2).reshape(-1, 32)[:cfg.NLOC]
        sxT = np.ascontiguousarray((hp_loc * dis2[sh, None]).T)
        idxall, _ = pack_idx(plans["B"][d], cfg)
        m = dict(table=tableB, sxT=sxT, idxall=idxall,
                 b2c=b2.reshape(32, 1), lw=lin_w, lb=lin_b.reshape(1, 1))
        for q in range(4):
            m[f"S{q}"] = plans["B"][d][q]["S"]
        return m

    with ThreadPoolExecutor(cfg.P) as exe:
        resB = list(exe.map(lambda d: runners[d][1](inputs_B(d)), range(cfg.P)))
    out = np.concatenate([np.asarray(r["outT"]).T for r in resB], axis=0)
    return out.astype(np.float32)


def kernel(x, edge_index, edge_logits, W1, b1, W2, b2, lin_w, lin_b):
    inputs = dict(x=x, edge_index=edge_index, edge_logits=edge_logits,
                  W1=W1, b1=b1, W2=W2, b2=b2, lin_w=lin_w, lin_b=lin_b)
    return run_two_phase(inputs, FULL)


# revision 15
# speedup vs baseline: 1.0175x; 1.0175x over previous
"""Trainium2 Bass kernel for a 2-layer edge-gated GCN (DiffGNNPlacement).

Math (reference, per layer):
    ew   = 0.5 + sigmoid(edge_logits)                  # [E]
    deg  = segsum(ew -> col) + 1                       # [N]
    dis  = deg^-1/2
    norm = dis[row] * ew * dis[col]                    # [E]
    out  = segsum(norm * (h@W)[row] -> col) + (h@W)*dis^2 + b

Aggregation commutes with the linear transform, so each phase aggregates the
raw features and applies the dense transform afterwards (phase A), or
aggregates pre-transformed features (phase B aggregates h1@W2, 32ch).

Device algorithm (per core, nodes sharded 12500/core, 2 programs):
  - per-edge rows gathered via SWDGE dma_gather with sub-256B elem sizes
    (128B in A, 64B in B); the feature tables are packed 2 resp. 4 nodes per
    256B stride cell, and the 4 streams per core (one per SWDGE queue) are
    keyed by (chunk, sub-offset) so the int16 gather indices address packed
    cells while the dma base encodes the sub-cell byte offset.
  - all gather indices are SBUF-resident from program start (one DMA), so
    the Q7 descriptor loops never stall on mid-stream index loads.
  - per tile: 128 gathered rows x S[128, 32] one-hot-times-norm bf16 matmul
    accumulated into a [CH, 512] PSUM window.
  - per window (transposed pipeline): z = psum + sxT; phase A: h =
    relu(W1^T z + b1), hp = W2^T h (32ch), transposed back via identity
    matmuls and written as the packed phase-B table; phase B: relu(z + b2),
    classifier head -> outT.
Host does structure-only planning (norms, sorting, one-hot S, index packing)
plus the inter-phase table relay, like the two-phase baseline.
"""

import os
import sys
import numpy as np
from contextlib import ExitStack

for _p in ("/opt/trn_rl_repo", "/root/.axon_site/_ro/trn_rl_repo"):
    if os.path.isdir(_p) and _p not in sys.path:
        sys.path.insert(0, _p)

import ml_dtypes

BF16 = np.dtype(ml_dtypes.bfloat16)


# ----------------------------------------------------------------- config ---
class Cfg:
    def __init__(self):
        self.N = 100000
        self.E = 1600000
        self.C = 64           # feature channels (phase A)
        self.H2 = 32          # phase B channels
        self.P = 8
        self.NLOC = self.N // self.P          # 12500
        self.W = 48           # S tile width (99.7% tile fill at 4 edges/col)
        self.WIN = 512        # PSUM window
        self.TCH = 32         # tiles per gather chunk
        self.NWIN = (self.NLOC + self.WIN - 1) // self.WIN
        # phase A: x packed 2 nodes / 256B cell -> 50000 cells, 2 chunks of
        # 25000 cells (int16), sub in {0,1}: queue = chunk*2 + sub
        self.A_CELLS = self.N // 2            # 50000
        self.A_CHUNK = 25000
        # phase B: h1p packed 4 rows / 256B group; per-shard padded rows
        self.TPP = 98                          # table rows per partition
        self.NLOC_PAD = 128 * self.TPP         # 12544
        self.B_ROWS = self.P * self.NLOC_PAD   # 100352
        self.B_CELLS = self.B_ROWS // 4        # 25088 (fits int16)


FULL = Cfg()


# ------------------------------------------------------- raw gather (sub-256B)
def dma_gather_raw(gp, out_ap, in_ap, idxs_ap, num_idxs, elem_size,
                   elem_step, queue_num):
    """nc.gpsimd.dma_gather minus the elem_size_bytes%256 assert (the
    non-transpose ucode/decode path supports arbitrary elem sizes; only the
    table stride must be a multiple of 256B)."""
    import concourse.mybir as mybir
    from concourse.bass import MemorySpace
    from concourse import ap_utils

    gp._assert_queue_num(queue_num)
    assert idxs_ap.dtype == mybir.dt.int16
    assert in_ap.space == MemorySpace.DRAM
    assert in_ap.dtype == out_ap.dtype
    assert idxs_ap.space == MemorySpace.SBUF
    assert out_ap.space == MemorySpace.SBUF
    assert ap_utils.ap_is_contiguous(in_ap.ap[1:])
    assert ap_utils.ap_is_contiguous(out_ap.ap[1:])
    assert ap_utils.ap_is_contiguous(idxs_ap.ap[1:])
    assert in_ap.ap[-1][1] == out_ap.ap[-1][1] == elem_size
    assert out_ap.ap[0][1] * out_ap.ap[1][1] == num_idxs  # num_idxs % 128 == 0
    assert in_ap.ap[0][0] == elem_step
    stride_bytes = elem_step * mybir.dt.size(in_ap.dtype)
    stride_bytes_256, rem = divmod(stride_bytes, 256)
    assert rem == 0 and 0 < stride_bytes_256 < 256

    _in_ap = gp.lower_ap_dma(in_ap, for_custom_bir_dma=True)
    _idxs_ap = gp.lower_ap(idxs_ap)
    _out_ap = gp.lower_ap(out_ap)
    return gp.add_instruction(
        mybir.InstDMAGatherAnt(
            name=gp.bass.get_next_instruction_name(),
            ins=[*_in_ap, _idxs_ap,
                 gp.lower_val_access(gp.to_reg(num_idxs))],
            outs=[_out_ap],
            transpose=False,
            num_idxs=num_idxs,
            elem_size=elem_size,
            stride_bytes_256=stride_bytes_256,
            gen_mode=0,
            single_packet=False,
            queue_num=queue_num,
            sbuf_tokens_per_rank=0,
            sbuf_free_dim_per_rank=0,
            sbuf_free_dim_pad_per_rank=0,
            sbuf_byte_offset=0,
        )
    )


# --------------------------------------------------------- host preprocess ---
def _sigmoid(x):
    return 0.5 * (np.tanh(0.5 * x) + 1.0)


def _cell_of_local(sl, cfg):
    """phase-B packed table row of local node sl (partition-major layout)."""
    return (sl % 128) * cfg.TPP + sl // 128


def _plan_stream(units, cols, vals, cfg):
    """Tile one sorted-by-col edge stream: 128-slot tiles, <=W col span.
    units: int16 gather indices (packed-cell units). Returns packed idx
    (16-part wrapped) + S arrays + window placement."""
    m = len(cols)
    starts, c0s = [], []
    i = 0
    while i < m:
        c0 = int(cols[i])
        jmax = min(i + 128, m)
        j = i + int(np.searchsorted(cols[i:jmax], c0 + cfg.W, side="left"))
        starts.append(i)
        c0s.append(c0)
        i = j
    T = len(c0s)
    starts_a = np.array(starts + [m], dtype=np.int64)
    c0s = np.array(c0s, dtype=np.int32)

    tile_of = np.repeat(np.arange(T), np.diff(starts_a))
    slot = np.arange(m) - starts_a[tile_of]
    idx16 = np.zeros((T, 128), np.int16)
    idx16[tile_of, slot] = units
    S = np.zeros((T, 128, cfg.W), np.float32)
    S[tile_of, slot, cols - c0s[tile_of]] = vals

    TCH = cfg.TCH
    nch = max(1, (T + TCH - 1) // TCH)
    Tp = nch * TCH
    flat = np.zeros(Tp * 128, np.int16)
    flat[: T * 128] = idx16.reshape(-1)
    # wrap: slot k*16+j -> [j, k]
    wrapped = flat.reshape(nch, TCH * 128 // 16, 16).transpose(0, 2, 1)
    idx_w = np.ascontiguousarray(wrapped)                 # [nch,16,TCH*8]
    Sp = np.zeros((Tp, 128, cfg.W), np.float32)
    Sp[:T] = S
    S_pk = np.ascontiguousarray(
        Sp.reshape(nch, TCH, 128, cfg.W).transpose(0, 2, 1, 3)).astype(BF16)

    win = c0s // cfg.WIN
    off = c0s - win * cfg.WIN
    return dict(T=T, nch=nch, idx=idx_w, S=S_pk, win=win, off=off)


def preprocess(edge_index, edge_logits, cfg=FULL):
    """Norms + per-device tile plans for both phases (pure numpy)."""
    N, NLOC = cfg.N, cfg.NLOC
    row = np.asarray(edge_index[0], dtype=np.int64)
    col = np.asarray(edge_index[1], dtype=np.int64)
    ew = (0.5 + _sigmoid(np.asarray(edge_logits, dtype=np.float32))).astype(np.float32)
    deg = np.bincount(col, weights=ew.astype(np.float64), minlength=N).astype(np.float32) + 1.0
    dis = deg ** -0.5
    norm = (dis[row] * ew * dis[col]).astype(np.float32)

    dev = col // NLOC
    lcol = (col - dev * NLOC).astype(np.int32)

    # phase A stream key: queue = (row//50000)*2 + row%2, unit = row//2 - chunk*25000
    qa = (row // (cfg.A_CHUNK * 2)) * 2 + (row % 2)
    ua = (row // 2) - (row // (cfg.A_CHUNK * 2)) * cfg.A_CHUNK
    # phase B: packed global row -> queue rB%4, unit rB//4
    ds = row // NLOC
    sl = row % NLOC
    rB = ds * cfg.NLOC_PAD + _cell_of_local(sl, cfg)
    qb = rB % 4
    ub = rB // 4
    assert ub.max() < 32768 and ua.max() < 32768

    plans = {"A": [], "B": []}
    for phase, q, u in (("A", qa, ua), ("B", qb, ub)):
        order = np.lexsort((lcol, q, dev))
        so_u, so_c, so_v = u[order].astype(np.int16), lcol[order], norm[order]
        so_d, so_q = dev[order], q[order]
        key = so_d * 4 + so_q
        bounds = np.searchsorted(key, np.arange(cfg.P * 4 + 1))
        for d in range(cfg.P):
            qplans = []
            for g in range(4):
                a, b = bounds[d * 4 + g], bounds[d * 4 + g + 1]
                qplans.append(_plan_stream(so_u[a:b], so_c[a:b], so_v[a:b], cfg))
            plans[phase].append(qplans)
    return plans, dis


def pack_idx(qplans, cfg):
    """Assemble the resident idx tile [128, NCHMAX*TCH*8] int16: queue q's
    wrapped indices replicated to partitions [32q,32q+16) and [32q+16,32q+32)."""
    nchmax = max(p["nch"] for p in qplans)
    width = nchmax * cfg.TCH * 8
    out = np.zeros((128, width), np.int16)
    for q, p in enumerate(qplans):
        flat = p["idx"].transpose(1, 0, 2).reshape(16, -1)  # [16, nch*TCH*8]
        out[32 * q:32 * q + 16, : flat.shape[1]] = flat
        out[32 * q + 16:32 * q + 32, : flat.shape[1]] = flat
    return out, nchmax


# ---------------------------------------------------------- program builder ---
def build_program(qplans, phase, cfg=FULL, name="gnn"):
    import concourse.bass as bass
    import concourse.mybir as mybir
    from concourse import bacc
    from concourse.tile import TileContext

    f32, bf16, i16 = mybir.dt.float32, mybir.dt.bfloat16, mybir.dt.int16
    W, WIN, TCH, NLOC = cfg.W, cfg.WIN, cfg.TCH, cfg.NLOC
    CH = cfg.C if phase == "A" else cfg.H2   # gathered/agg channels

    nc = bacc.Bacc("TRN2", enable_partition_id=False,
                   target_bir_lowering=False, name=name,
                   num_swdge_queues=4)

    if phase == "A":
        table = nc.dram_tensor("table", [cfg.A_CELLS, 128], bf16, kind="ExternalInput")
    else:
        table = nc.dram_tensor("table", [cfg.B_CELLS, 128], bf16, kind="ExternalInput")
    sxT_dr = nc.dram_tensor("sxT", [CH, NLOC], f32, kind="ExternalInput")
    _, nchmax = pack_idx(qplans, cfg)
    idx_dr = nc.dram_tensor("idxall", [128, nchmax * TCH * 8], i16, kind="ExternalInput")
    S_dr = []
    for q in range(4):
        p = qplans[q]
        S_dr.append(nc.dram_tensor(f"S{q}", list(p["S"].shape), bf16,
                                   kind="ExternalInput"))
    if phase == "A":
        W1_dr = nc.dram_tensor("W1", [64, 64], f32, kind="ExternalInput")
        b1_dr = nc.dram_tensor("b1c", [64, 1], f32, kind="ExternalInput")
        W2_dr = nc.dram_tensor("W2p", [64, 32], f32, kind="ExternalInput")
        id_dr = nc.dram_tensor("ident", [32, 32], bf16, kind="ExternalInput")
        h_out = nc.dram_tensor("h_out", [cfg.NLOC_PAD, 32], bf16, kind="ExternalOutput")
        dst3 = h_out.rearrange("(p t) c -> p t c", p=128)
    else:
        b2_dr = nc.dram_tensor("b2c", [32, 1], f32, kind="ExternalInput")
        lw_dr = nc.dram_tensor("lw", [32, 1], f32, kind="ExternalInput")
        lb_dr = nc.dram_tensor("lb", [1, 1], f32, kind="ExternalInput")
        outT = nc.dram_tensor("outT", [2, NLOC], f32, kind="ExternalOutput")

    # per-window tile lists: (q, t, off, s_lo, weff); straddling tiles split
    win_tiles = [[] for _ in range(cfg.NWIN)]
    for q in range(4):
        p = qplans[q]
        for t in range(p["T"]):
            w = int(p["win"][t])
            off = int(p["off"][t])
            wlen = min(WIN, NLOC - w * WIN)
            w1 = min(W, WIN - off)
            win_tiles[w].append((q, t, off, 0, min(w1, wlen - off)))
            if W > w1 and w + 1 < cfg.NWIN:
                wlen2 = min(WIN, NLOC - (w + 1) * WIN)
                win_tiles[w + 1].append((q, t, 0, w1, min(W - w1, wlen2)))

    with TileContext(nc) as tc, ExitStack() as ex:
        cpool = ex.enter_context(tc.tile_pool(name="consts", bufs=1))
        zpool = ex.enter_context(tc.tile_pool(name="z", bufs=3))
        sxpool = ex.enter_context(tc.tile_pool(name="sx", bufs=3))
        gpools = [ex.enter_context(tc.tile_pool(name=f"gat{q}", bufs=4)) for q in range(4)]
        spools = [ex.enter_context(tc.tile_pool(name=f"s{q}", bufs=4)) for q in range(4)]
        ppool = ex.enter_context(tc.tile_pool(name="psagg", bufs=2, space="PSUM"))
        if phase == "A":
            pstpool = ex.enter_context(tc.tile_pool(name="psd", bufs=2, space="PSUM"))
            hppool = ex.enter_context(tc.tile_pool(name="psh", bufs=2, space="PSUM"))
            tppool = ex.enter_context(tc.tile_pool(name="pst", bufs=2, space="PSUM"))
            htpool = ex.enter_context(tc.tile_pool(name="ht", bufs=2))
            hptpool = ex.enter_context(tc.tile_pool(name="hpt", bufs=2))
            hspool = ex.enter_context(tc.tile_pool(name="hs", bufs=3))
        else:
            plpool = ex.enter_context(tc.tile_pool(name="psl", bufs=2, space="PSUM"))
            htpool = ex.enter_context(tc.tile_pool(name="ht", bufs=2))
            opool = ex.enter_context(tc.tile_pool(name="ot", bufs=3))

        # resident gather indices, loaded in two stages so chunk-0/1 gathers
        # start without waiting for the full index image
        head = min(2, nchmax) * TCH * 8
        idx_sb0 = cpool.tile([128, head], i16)
        nc.sync.dma_start(out=idx_sb0[:, :], in_=idx_dr[:, :head])
        idx_sb1 = None
        if nchmax > 2:
            idx_sb1 = cpool.tile([128, (nchmax - 2) * TCH * 8], i16)

        def idx_slice(ch, ntl):
            lo, hi = ch * TCH * 8, (ch * TCH + ntl) * 8
            if hi <= head:
                return idx_sb0[:, lo:hi]
            return idx_sb1[:, lo - head:hi - head]

        # last two chunks stay live per queue: a window-straddling tile's
        # continuation may be processed after the next chunk was entered
        # (pool bufs=4 keeps both chunks' tiles valid)
        cur = [{} for _ in range(4)]

        def table_slice(q):
            if phase == "A":
                g, s = q // 2, q % 2
                return table[g * cfg.A_CHUNK:(g + 1) * cfg.A_CHUNK,
                             s * 64:(s + 1) * 64]
            return table[:, (q % 4) * 32:(q % 4) * 32 + 32]

        def ensure_chunk(q, ch):
            st = cur[q]
            if ch in st:
                return st[ch]
            p = qplans[q]
            ntl = min(TCH, p["T"] - ch * TCH)
            nid = ntl * 128
            gb = gpools[q].tile([128, TCH, CH], bf16, tag="g")
            dma_gather_raw(
                nc.gpsimd, gb[:, :ntl, :], table_slice(q),
                idx_slice(ch, ntl),
                nid, CH, 128, q)
            sb = spools[q].tile([128, TCH, W], bf16, tag="s")
            nc.scalar.dma_start(out=sb[:, :ntl, :], in_=S_dr[q][ch, :, :ntl, :])
            st[ch] = dict(gb=gb, sb=sb)
            for old in [k for k in st if k < ch - 1]:
                del st[old]
            return st[ch]

        for q in range(4):
            ensure_chunk(q, 0)
        if idx_sb1 is not None:
            nc.sync.dma_start(out=idx_sb1[:, :], in_=idx_dr[:, head:])

        # ---- constants
        zrow = cpool.tile([1, WIN], bf16)
        nc.vector.memset(zrow[:, :], 0.0)
        if phase == "A":
            W1_sb = cpool.tile([64, 64], f32)
            nc.sync.dma_start(out=W1_sb[:, :], in_=W1_dr[:, :])
            b1_sb = cpool.tile([64, 1], f32)
            nc.sync.dma_start(out=b1_sb[:, :], in_=b1_dr[:, :])
            W2_sb = cpool.tile([64, 32], f32)
            nc.sync.dma_start(out=W2_sb[:, :], in_=W2_dr[:, :])
            id_sb = cpool.tile([32, 32], bf16)
            nc.sync.dma_start(out=id_sb[:, :], in_=id_dr[:, :])
        else:
            b2_sb = cpool.tile([32, 1], f32)
            nc.sync.dma_start(out=b2_sb[:, :], in_=b2_dr[:, :])
            lw_sb = cpool.tile([32, 1], f32)
            nc.sync.dma_start(out=lw_sb[:, :], in_=lw_dr[:, :])
            lb_sb = cpool.tile([1, 1], f32)
            nc.sync.dma_start(out=lb_sb[:, :], in_=lb_dr[:, :])
            nlb = cpool.tile([1, 1], f32)
            nc.scalar.mul(nlb[:, :], lb_sb[:, :], -1.0)

        for w in range(cfg.NWIN):
            wlen = min(WIN, NLOC - w * WIN)
            ps = ppool.tile([CH, WIN], f32)
            nc.tensor.matmul(ps[:, :wlen], lhsT=zrow[:, :CH], rhs=zrow[:, :wlen],
                             start=True, stop=False)
            for q, t, off, s_lo, weff in win_tiles[w]:
                st = ensure_chunk(q, t // TCH)
                tp = t % TCH
                nc.tensor.matmul(
                    ps[:, off:off + weff],
                    lhsT=st["gb"][:, tp, :CH],
                    rhs=st["sb"][:, tp, s_lo:s_lo + weff],
                    start=False, stop=False,
                    skip_group_check=True,
                )
            nc.tensor.matmul(ps[:, :wlen], lhsT=zrow[:, :CH], rhs=zrow[:, :wlen],
                             start=False, stop=True)
            sxw = sxpool.tile([CH, WIN], f32, tag="sx")
            nc.sync.dma_start(out=sxw[:, :wlen],
                              in_=sxT_dr[:, w * WIN:w * WIN + wlen])
            zw = zpool.tile([CH, WIN], f32, tag="z")
            nc.vector.tensor_tensor(out=zw[:, :wlen], in0=ps[:, :wlen],
                                    in1=sxw[:, :wlen], op=mybir.AluOpType.add)

            if phase == "A":
                pst = pstpool.tile([64, WIN], f32)
                nc.tensor.matmul(pst[:, :wlen], lhsT=W1_sb[:, :],
                                 rhs=zw[:, :wlen], start=True, stop=True)
                ht = htpool.tile([64, WIN], f32, tag="ht")
                nc.scalar.activation(ht[:, :wlen], pst[:, :wlen],
                                     mybir.ActivationFunctionType.Relu,
                                     bias=b1_sb[:, :])
                hp = hppool.tile([32, WIN], f32)
                nc.tensor.matmul(hp[:, :wlen], lhsT=W2_sb[:, :],
                                 rhs=ht[:, :wlen], start=True, stop=True)
                hpT = hptpool.tile([32, WIN], bf16, tag="hpT")
                nc.vector.tensor_copy(hpT[:, :wlen], hp[:, :wlen])
                nck = (wlen + 127) // 128
                hs = hspool.tile([128, 4, 32], bf16, tag="hs")
                for kk in range(nck):
                    mrow = min(128, wlen - kk * 128)
                    tpp = tppool.tile([128, 32], f32)
                    nc.tensor.matmul(tpp[:mrow, :],
                                     lhsT=hpT[:, kk * 128:kk * 128 + mrow],
                                     rhs=id_sb[:, :], start=True, stop=True)
                    nc.vector.tensor_copy(hs[:mrow, kk, :], tpp[:mrow, :])
                nc.sync.dma_start(out=dst3[:, w * 4:w * 4 + nck, :],
                                  in_=hs[:, :nck, :])
            else:
                ht2 = htpool.tile([32, WIN], f32, tag="ht2")
                nc.scalar.activation(ht2[:, :wlen], zw[:, :wlen],
                                     mybir.ActivationFunctionType.Relu,
                                     bias=b2_sb[:, :])
                psl = plpool.tile([1, WIN], f32)
                nc.tensor.matmul(psl[:, :wlen], lhsT=lw_sb[:, :],
                                 rhs=ht2[:, :wlen], start=True, stop=True)
                otn = opool.tile([1, WIN], f32, tag="otn")
                otp = opool.tile([1, WIN], f32, tag="otp")
                nc.scalar.activation(otn[:, :wlen], psl[:, :wlen],
                                     mybir.ActivationFunctionType.Identity,
                                     bias=nlb[:, :], scale=-1.0)
                nc.scalar.activation(otp[:, :wlen], psl[:, :wlen],
                                     mybir.ActivationFunctionType.Identity,
                                     bias=lb_sb[:, :], scale=1.0)
                nc.sync.dma_start(out=outT[0:1, w * WIN:w * WIN + wlen],
                                  in_=otn[:, :wlen])
                nc.sync.dma_start(out=outT[1:2, w * WIN:w * WIN + wlen],
                                  in_=otp[:, :wlen])

    nc.compile()
    return nc


# ------------------------------------------------------------------ runner ---
def make_runner(nc, device):
    """Single-core jit runner pinned to one device, reusable across calls."""
    import jax
    import concourse.mybir as mybir
    from concourse import bass2jax

    bass2jax.install_neuronx_cc_hook()

    in_names, out_names, out_avals, zero_shapes = [], [], [], []
    for alloc in nc.m.functions[0].allocations:
        if not isinstance(alloc, mybir.MemoryLocationSet):
            continue
        nm = alloc.memorylocations[0].name
        if alloc.kind == "ExternalInput":
            in_names.append(nm)
        elif alloc.kind == "ExternalOutput":
            shape = tuple(alloc.tensor_shape)
            dtype = mybir.dt.np(alloc.dtype)
            out_names.append(nm)
            out_avals.append(jax.core.ShapedArray(shape, dtype))
            zero_shapes.append((shape, dtype))
    n_params = len(in_names)
    all_in_names = in_names + out_names
    donate = tuple(range(n_params, n_params + len(out_names)))

    def _body(*args):
        outs = bass2jax._bass_exec_p.bind(
            *args,
            out_avals=tuple(out_avals),
            in_names=tuple(all_in_names),
            out_names=tuple(out_names),
            lowering_input_output_aliases=(),
            sim_require_finite=True,
            sim_require_nnan=True,
            nc=nc,
        )
        return tuple(outs)

    jitted = jax.jit(_body, donate_argnums=donate, keep_unused=True)

    def run(in_map):
        args = [jax.device_put(np.asarray(in_map[nm]), device) for nm in in_names]
        zeros = [jax.device_put(np.zeros(s, d), device) for s, d in zero_shapes]
        outs = jitted(*args, *zeros)
        return {nm: outs[i] for i, nm in enumerate(out_names)}

    return run


# ---------------------------------------------------------------- kernel() ---
_CACHE = {}


def _get_runners(plans, cfg):
    import jax
    from concurrent.futures import ThreadPoolExecutor
    key = "runners"
    if key in _CACHE:
        return _CACHE[key]
    devices = jax.devices()[:cfg.P]

    def build_pair(d):
        ncA = build_program(plans["A"][d], "A", cfg, name=f"gnnA_d{d}")
        ncB = build_program(plans["B"][d], "B", cfg, name=f"gnnB_d{d}")
        return (make_runner(ncA, devices[d]), make_runner(ncB, devices[d]))

    with ThreadPoolExecutor(4) as exe:
        runners = list(exe.map(build_pair, range(cfg.P)))
    _CACHE[key] = runners
    return runners


def run_two_phase(inputs, cfg=FULL):
    from concurrent.futures import ThreadPoolExecutor

    x = np.asarray(inputs["x"], np.float32)
    W1 = np.asarray(inputs["W1"], np.float32)
    b1 = np.asarray(inputs["b1"], np.float32)
    W2 = np.asarray(inputs["W2"], np.float32)
    b2 = np.asarray(inputs["b2"], np.float32)
    lin_w = np.asarray(inputs["lin_w"], np.float32)
    lin_b = np.asarray(inputs["lin_b"], np.float32)

    plans, dis = preprocess(inputs["edge_index"], inputs["edge_logits"], cfg)
    dis2 = (dis * dis).astype(np.float32)
    runners = _get_runners(plans, cfg)

    ident = np.eye(32, dtype=np.float32).astype(BF16)
    x_pack = np.ascontiguousarray(x.astype(BF16).reshape(cfg.A_CELLS, 128))

    def inputs_A(d):
        sh = slice(d * cfg.NLOC, (d + 1) * cfg.NLOC)
        sxT = np.ascontiguousarray((x[sh] * dis2[sh, None]).T)
        idxall, _ = pack_idx(plans["A"][d], cfg)
        m = dict(table=x_pack, sxT=sxT, idxall=idxall,
                 W1=W1, b1c=b1.reshape(64, 1), W2p=W2, ident=ident)
        for q in range(4):
            m[f"S{q}"] = plans["A"][d][q]["S"]
        return m

    with ThreadPoolExecutor(cfg.P) as exe:
        resA = list(exe.map(lambda d: runners[d][0](inputs_A(d)), range(cfg.P)))
    # assemble phase-B table: concat per-shard packed h1p rows
    h_shards = [np.asarray(r["h_out"]) for r in resA]       # [12544, 32] bf16
    tableB = np.ascontiguousarray(
        np.concatenate(h_shards, axis=0).reshape(cfg.B_CELLS, 128))

    def inputs_B(d):
        sh = slice(d * cfg.NLOC, (d + 1) * cfg.NLOC)
        # local h1p in node order: cell(p,t) = p*TPP+t holds node t*128+p
        hp_loc = h_shards[d].astype(np.float32).reshape(128, cfg.TPP, 32)
        hp_loc = hp_loc.transpose(1, 0, 2).reshape(-1, 32)[:cfg.NLOC]
        sxT = np.ascontiguousarray((hp_loc * dis2[sh, None]).T)
        idxall, _ = pack_idx(plans["B"][d], cfg)
        m = dict(table=tableB, sxT=sxT, idxall=idxall,
                 b2c=b2.reshape(32, 1), lw=lin_w, lb=lin_b.reshape(1, 1))
        for q in range(4):
            m[f"S{q}"] = plans["B"][d][q]["S"]
        return m

    with ThreadPoolExecutor(cfg.P) as exe:
        resB = list(exe.map(lambda d: runners[d][1](inputs_B(d)), range(cfg.P)))
    out = np.concatenate([np.asarray(r["outT"]).T for r in resB], axis=0)
    return out.astype(np.float32)


def kernel(x, edge_index, edge_logits, W1, b1, W2, b2, lin_w, lin_b):
    inputs = dict(x=x, edge_index=edge_index, edge_logits=edge_logits,
                  W1=W1, b1=b1, W2=W2, b2=b2, lin_w=lin_w, lin_b=lin_b)
    return run_two_phase(inputs, FULL)


# revision 23
# speedup vs baseline: 1.0209x; 1.0033x over previous
"""Trainium2 Bass kernel for a 2-layer edge-gated GCN (DiffGNNPlacement).

Math (reference, per layer):
    ew   = 0.5 + sigmoid(edge_logits)                  # [E]
    deg  = segsum(ew -> col) + 1                       # [N]
    dis  = deg^-1/2
    norm = dis[row] * ew * dis[col]                    # [E]
    out  = segsum(norm * (h@W)[row] -> col) + (h@W)*dis^2 + b

Aggregation commutes with the linear transform, so each phase aggregates the
raw features and applies the dense transform afterwards (phase A), or
aggregates pre-transformed features (phase B aggregates h1@W2, 32ch).

Device algorithm (per core, nodes sharded 12500/core, 2 programs):
  - per-edge rows gathered via SWDGE dma_gather with sub-256B elem sizes
    (128B in A, 64B in B); the feature tables are packed 2 resp. 4 nodes per
    256B stride cell, and the 4 streams per core (one per SWDGE queue) are
    keyed by (chunk, sub-offset) so the int16 gather indices address packed
    cells while the dma base encodes the sub-cell byte offset.
  - all gather indices are SBUF-resident from program start (one DMA), so
    the Q7 descriptor loops never stall on mid-stream index loads.
  - per tile: 128 gathered rows x S[128, 32] one-hot-times-norm bf16 matmul
    accumulated into a [CH, 512] PSUM window.
  - per window (transposed pipeline): z = psum + sxT; phase A: h =
    relu(W1^T z + b1), hp = W2^T h (32ch), transposed back via identity
    matmuls and written as the packed phase-B table; phase B: relu(z + b2),
    classifier head -> outT.
Host does structure-only planning (norms, sorting, one-hot S, index packing)
plus the inter-phase table relay, like the two-phase baseline.
"""

import os
import sys
import numpy as np
from contextlib import ExitStack

for _p in ("/opt/trn_rl_repo", "/root/.axon_site/_ro/trn_rl_repo"):
    if os.path.isdir(_p) and _p not in sys.path:
        sys.path.insert(0, _p)

import ml_dtypes

BF16 = np.dtype(ml_dtypes.bfloat16)


# ----------------------------------------------------------------- config ---
class Cfg:
    def __init__(self):
        self.N = 100000
        self.E = 1600000
        self.C = 64           # feature channels (phase A)
        self.H2 = 32          # phase B channels
        self.P = 8
        self.NLOC = self.N // self.P          # 12500
        self.W = 48           # S tile width (99.7% tile fill at 4 edges/col)
        self.WIN = 512        # PSUM window
        self.TCH = 32         # tiles per gather chunk
        self.NWIN = (self.NLOC + self.WIN - 1) // self.WIN
        # phase A: x packed 2 nodes / 256B cell -> 50000 cells, 2 chunks of
        # 25000 cells (int16), sub in {0,1}: queue = chunk*2 + sub
        self.A_CELLS = self.N // 2            # 50000
        self.A_CHUNK = 25000
        # phase B: h1p (node-order) packed 4 rows / 256B group
        self.B_CELLS = self.N // 4             # 25000 (fits int16)


FULL = Cfg()


# ------------------------------------------------------- raw gather (sub-256B)
def dma_gather_raw(gp, out_ap, in_ap, idxs_ap, num_idxs, elem_size,
                   elem_step, queue_num):
    """nc.gpsimd.dma_gather minus the elem_size_bytes%256 assert (the
    non-transpose ucode/decode path supports arbitrary elem sizes; only the
    table stride must be a multiple of 256B)."""
    import concourse.mybir as mybir
    from concourse.bass import MemorySpace
    from concourse import ap_utils

    gp._assert_queue_num(queue_num)
    assert idxs_ap.dtype == mybir.dt.int16
    assert in_ap.space == MemorySpace.DRAM
    assert in_ap.dtype == out_ap.dtype
    assert idxs_ap.space == MemorySpace.SBUF
    assert out_ap.space == MemorySpace.SBUF
    assert ap_utils.ap_is_contiguous(in_ap.ap[1:])
    assert ap_utils.ap_is_contiguous(out_ap.ap[1:])
    assert ap_utils.ap_is_contiguous(idxs_ap.ap[1:])
    assert in_ap.ap[-1][1] == out_ap.ap[-1][1] == elem_size
    assert out_ap.ap[0][1] * out_ap.ap[1][1] == num_idxs  # num_idxs % 128 == 0
    assert in_ap.ap[0][0] == elem_step
    stride_bytes = elem_step * mybir.dt.size(in_ap.dtype)
    stride_bytes_256, rem = divmod(stride_bytes, 256)
    assert rem == 0 and 0 < stride_bytes_256 < 256

    _in_ap = gp.lower_ap_dma(in_ap, for_custom_bir_dma=True)
    _idxs_ap = gp.lower_ap(idxs_ap)
    _out_ap = gp.lower_ap(out_ap)
    return gp.add_instruction(
        mybir.InstDMAGatherAnt(
            name=gp.bass.get_next_instruction_name(),
            ins=[*_in_ap, _idxs_ap,
                 gp.lower_val_access(gp.to_reg(num_idxs))],
            outs=[_out_ap],
            transpose=False,
            num_idxs=num_idxs,
            elem_size=elem_size,
            stride_bytes_256=stride_bytes_256,
            gen_mode=0,
            single_packet=False,
            queue_num=queue_num,
            sbuf_tokens_per_rank=0,
            sbuf_free_dim_per_rank=0,
            sbuf_free_dim_pad_per_rank=0,
            sbuf_byte_offset=0,
        )
    )


# --------------------------------------------------------- host preprocess ---
def _sigmoid(x):
    return 0.5 * (np.tanh(0.5 * x) + 1.0)


def _plan_stream(units, cols, vals, cfg):
    """Tile one sorted-by-col edge stream: 128-slot tiles, <=W col span.
    units: int16 gather indices (packed-cell units). Returns packed idx
    (16-part wrapped) + S arrays + window placement."""
    m = len(cols)
    starts, c0s = [], []
    i = 0
    while i < m:
        c0 = int(cols[i])
        jmax = min(i + 128, m)
        j = i + int(np.searchsorted(cols[i:jmax], c0 + cfg.W, side="left"))
        starts.append(i)
        c0s.append(c0)
        i = j
    T = len(c0s)
    starts_a = np.array(starts + [m], dtype=np.int64)
    c0s = np.array(c0s, dtype=np.int32)

    tile_of = np.repeat(np.arange(T), np.diff(starts_a))
    slot = np.arange(m) - starts_a[tile_of]
    idx16 = np.zeros((T, 128), np.int16)
    idx16[tile_of, slot] = units
    S = np.zeros((T, 128, cfg.W), np.float32)
    S[tile_of, slot, cols - c0s[tile_of]] = vals

    TCH = cfg.TCH
    nch = max(1, (T + TCH - 1) // TCH)
    Tp = nch * TCH
    flat = np.zeros(Tp * 128, np.int16)
    flat[: T * 128] = idx16.reshape(-1)
    # wrap: slot k*16+j -> [j, k]
    wrapped = flat.reshape(nch, TCH * 128 // 16, 16).transpose(0, 2, 1)
    idx_w = np.ascontiguousarray(wrapped)                 # [nch,16,TCH*8]
    Sp = np.zeros((Tp, 128, cfg.W), np.float32)
    Sp[:T] = S
    S_pk = np.ascontiguousarray(
        Sp.reshape(nch, TCH, 128, cfg.W).transpose(0, 2, 1, 3)).astype(BF16)

    win = c0s // cfg.WIN
    off = c0s - win * cfg.WIN
    return dict(T=T, nch=nch, idx=idx_w, S=S_pk, win=win, off=off)


def preprocess(edge_index, edge_logits, cfg=FULL):
    """Norms + per-device tile plans for both phases (pure numpy)."""
    N, NLOC = cfg.N, cfg.NLOC
    row = np.asarray(edge_index[0], dtype=np.int64)
    col = np.asarray(edge_index[1], dtype=np.int64)
    ew = (0.5 + _sigmoid(np.asarray(edge_logits, dtype=np.float32))).astype(np.float32)
    deg = np.bincount(col, weights=ew.astype(np.float64), minlength=N).astype(np.float32) + 1.0
    dis = deg ** -0.5
    norm = (dis[row] * ew * dis[col]).astype(np.float32)

    dev = col // NLOC
    lcol = (col - dev * NLOC).astype(np.int32)

    # phase A stream key: queue = (row//50000)*2 + row%2, unit = row//2 - chunk*25000
    qa = (row // (cfg.A_CHUNK * 2)) * 2 + (row % 2)
    ua = (row // 2) - (row // (cfg.A_CHUNK * 2)) * cfg.A_CHUNK
    # phase B: h1p table in plain node order -> queue row%4, unit row//4
    qb = row % 4
    ub = row // 4
    assert ub.max() < 32768 and ua.max() < 32768

    plans = {"A": [], "B": []}
    for phase, q, u in (("A", qa, ua), ("B", qb, ub)):
        order = np.lexsort((lcol, q, dev))
        so_u, so_c, so_v = u[order].astype(np.int16), lcol[order], norm[order]
        so_d, so_q = dev[order], q[order]
        key = so_d * 4 + so_q
        bounds = np.searchsorted(key, np.arange(cfg.P * 4 + 1))
        for d in range(cfg.P):
            qplans = []
            for g in range(4):
                a, b = bounds[d * 4 + g], bounds[d * 4 + g + 1]
                qplans.append(_plan_stream(so_u[a:b], so_c[a:b], so_v[a:b], cfg))
            plans[phase].append(qplans)
    return plans, dis


def pack_idx(qplans, cfg):
    """Assemble the resident idx tile [128, NCHMAX*TCH*8] int16: queue q's
    wrapped indices replicated to partitions [32q,32q+16) and [32q+16,32q+32)."""
    nchmax = max(p["nch"] for p in qplans)
    width = nchmax * cfg.TCH * 8
    out = np.zeros((128, width), np.int16)
    for q, p in enumerate(qplans):
        flat = p["idx"].transpose(1, 0, 2).reshape(16, -1)  # [16, nch*TCH*8]
        out[32 * q:32 * q + 16, : flat.shape[1]] = flat
        out[32 * q + 16:32 * q + 32, : flat.shape[1]] = flat
    return out, nchmax


# ---------------------------------------------------------- program builder ---
def build_program(qplans, phase, cfg=FULL, name="gnn"):
    import concourse.bass as bass
    import concourse.mybir as mybir
    from concourse import bacc
    from concourse.tile import TileContext

    f32, bf16, i16 = mybir.dt.float32, mybir.dt.bfloat16, mybir.dt.int16
    W, WIN, TCH, NLOC = cfg.W, cfg.WIN, cfg.TCH, cfg.NLOC
    CH = cfg.C if phase == "A" else cfg.H2   # gathered/agg channels

    nc = bacc.Bacc("TRN2", enable_partition_id=False,
                   target_bir_lowering=False, name=name,
                   num_swdge_queues=4)

    if phase == "A":
        table = nc.dram_tensor("table", [cfg.A_CELLS, 128], bf16, kind="ExternalInput")
    else:
        table = nc.dram_tensor("table", [cfg.B_CELLS, 128], bf16, kind="ExternalInput")
    sxT_dr = nc.dram_tensor("sxT", [CH, NLOC], f32, kind="ExternalInput")
    _, nchmax = pack_idx(qplans, cfg)
    idx_dr = nc.dram_tensor("idxall", [128, nchmax * TCH * 8], i16, kind="ExternalInput")
    S_dr = []
    for q in range(4):
        p = qplans[q]
        S_dr.append(nc.dram_tensor(f"S{q}", list(p["S"].shape), bf16,
                                   kind="ExternalInput"))
    if phase == "A":
        W1_dr = nc.dram_tensor("W1", [64, 64], f32, kind="ExternalInput")
        b1_dr = nc.dram_tensor("b1c", [64, 1], f32, kind="ExternalInput")
        W2_dr = nc.dram_tensor("W2p", [64, 32], f32, kind="ExternalInput")
        # h1p^T shard; the host transposes/packs it into the phase-B table
        h_out = nc.dram_tensor("h_out", [32, NLOC], bf16, kind="ExternalOutput")
    else:
        b2_dr = nc.dram_tensor("b2c", [32, 1], f32, kind="ExternalInput")
        lw_dr = nc.dram_tensor("lw", [32, 1], f32, kind="ExternalInput")
        lb_dr = nc.dram_tensor("lb", [1, 1], f32, kind="ExternalInput")
        outT = nc.dram_tensor("outT", [2, NLOC], f32, kind="ExternalOutput")

    # per-window tile lists: (q, t, off, s_lo, weff); straddling tiles split
    win_tiles = [[] for _ in range(cfg.NWIN)]
    for q in range(4):
        p = qplans[q]
        for t in range(p["T"]):
            w = int(p["win"][t])
            off = int(p["off"][t])
            wlen = min(WIN, NLOC - w * WIN)
            w1 = min(W, WIN - off)
            win_tiles[w].append((q, t, off, 0, min(w1, wlen - off)))
            if W > w1 and w + 1 < cfg.NWIN:
                wlen2 = min(WIN, NLOC - (w + 1) * WIN)
                win_tiles[w + 1].append((q, t, 0, w1, min(W - w1, wlen2)))

    with TileContext(nc) as tc, ExitStack() as ex:
        cpool = ex.enter_context(tc.tile_pool(name="consts", bufs=1))
        zpool = ex.enter_context(tc.tile_pool(name="z", bufs=3))
        sxpool = ex.enter_context(tc.tile_pool(name="sx", bufs=3))
        gpools = [ex.enter_context(tc.tile_pool(name=f"gat{q}", bufs=4)) for q in range(4)]
        spools = [ex.enter_context(tc.tile_pool(name=f"s{q}", bufs=4)) for q in range(4)]
        ppool = ex.enter_context(tc.tile_pool(name="psagg", bufs=2, space="PSUM"))
        if phase == "A":
            pstpool = ex.enter_context(tc.tile_pool(name="psd", bufs=2, space="PSUM"))
            hppool = ex.enter_context(tc.tile_pool(name="psh", bufs=2, space="PSUM"))
            htpool = ex.enter_context(tc.tile_pool(name="ht", bufs=2))
            hptpool = ex.enter_context(tc.tile_pool(name="hpt", bufs=3))
        else:
            plpool = ex.enter_context(tc.tile_pool(name="psl", bufs=2, space="PSUM"))
            htpool = ex.enter_context(tc.tile_pool(name="ht", bufs=2))
            opool = ex.enter_context(tc.tile_pool(name="ot", bufs=3))

        # resident gather indices, loaded in two stages so chunk-0/1 gathers
        # start without waiting for the full index image
        head = min(2, nchmax) * TCH * 8
        idx_sb0 = cpool.tile([128, head], i16)
        nc.sync.dma_start(out=idx_sb0[:, :], in_=idx_dr[:, :head])
        idx_sb1 = None
        if nchmax > 2:
            idx_sb1 = cpool.tile([128, (nchmax - 2) * TCH * 8], i16)

        def idx_slice(ch, ntl):
            lo, hi = ch * TCH * 8, (ch * TCH + ntl) * 8
            if hi <= head:
                return idx_sb0[:, lo:hi]
            return idx_sb1[:, lo - head:hi - head]

        # last two chunks stay live per queue: a window-straddling tile's
        # continuation may be processed after the next chunk was entered
        # (pool bufs=4 keeps both chunks' tiles valid)
        cur = [{} for _ in range(4)]

        def table_slice(q):
            if phase == "A":
                g, s = q // 2, q % 2
                return table[g * cfg.A_CHUNK:(g + 1) * cfg.A_CHUNK,
                             s * 64:(s + 1) * 64]
            return table[:, (q % 4) * 32:(q % 4) * 32 + 32]

        def ensure_chunk(q, ch):
            st = cur[q]
            if ch in st:
                return st[ch]
            p = qplans[q]
            ntl = min(TCH, p["T"] - ch * TCH)
            nid = ntl * 128
            gb = gpools[q].tile([128, TCH, CH], bf16, tag="g")
            dma_gather_raw(
                nc.gpsimd, gb[:, :ntl, :], table_slice(q),
                idx_slice(ch, ntl),
                nid, CH, 128, q)
            sb = spools[q].tile([128, TCH, W], bf16, tag="s")
            nc.scalar.dma_start(out=sb[:, :ntl, :], in_=S_dr[q][ch, :, :ntl, :])
            st[ch] = dict(gb=gb, sb=sb)
            for old in [k for k in st if k < ch - 1]:
                del st[old]
            return st[ch]

        for q in range(4):
            ensure_chunk(q, 0)
        if idx_sb1 is not None:
            nc.sync.dma_start(out=idx_sb1[:, :], in_=idx_dr[:, head:])

        # ---- constants
        zrow = cpool.tile([1, WIN], bf16)
        nc.vector.memset(zrow[:, :], 0.0)
        if phase == "A":
            W1_sb = cpool.tile([64, 64], f32)
            nc.sync.dma_start(out=W1_sb[:, :], in_=W1_dr[:, :])
            b1_sb = cpool.tile([64, 1], f32)
            nc.sync.dma_start(out=b1_sb[:, :], in_=b1_dr[:, :])
            W2_sb = cpool.tile([64, 32], f32)
            nc.sync.dma_start(out=W2_sb[:, :], in_=W2_dr[:, :])
        else:
            b2_sb = cpool.tile([32, 1], f32)
            nc.sync.dma_start(out=b2_sb[:, :], in_=b2_dr[:, :])
            lw_sb = cpool.tile([32, 1], f32)
            nc.sync.dma_start(out=lw_sb[:, :], in_=lw_dr[:, :])
            lb_sb = cpool.tile([1, 1], f32)
            nc.sync.dma_start(out=lb_sb[:, :], in_=lb_dr[:, :])
            nlb = cpool.tile([1, 1], f32)
            nc.scalar.mul(nlb[:, :], lb_sb[:, :], -1.0)

        for w in range(cfg.NWIN):
            wlen = min(WIN, NLOC - w * WIN)
            ps = ppool.tile([CH, WIN], f32)
            nc.tensor.matmul(ps[:, :wlen], lhsT=zrow[:, :CH], rhs=zrow[:, :wlen],
                             start=True, stop=False)
            for q, t, off, s_lo, weff in win_tiles[w]:
                st = ensure_chunk(q, t // TCH)
                tp = t % TCH
                nc.tensor.matmul(
                    ps[:, off:off + weff],
                    lhsT=st["gb"][:, tp, :CH],
                    rhs=st["sb"][:, tp, s_lo:s_lo + weff],
                    start=False, stop=False,
                    skip_group_check=True,
                )
            nc.tensor.matmul(ps[:, :wlen], lhsT=zrow[:, :CH], rhs=zrow[:, :wlen],
                             start=False, stop=True)
            sxw = sxpool.tile([CH, WIN], f32, tag="sx")
            nc.sync.dma_start(out=sxw[:, :wlen],
                              in_=sxT_dr[:, w * WIN:w * WIN + wlen])
            zw = zpool.tile([CH, WIN], f32, tag="z")
            nc.vector.tensor_tensor(out=zw[:, :wlen], in0=ps[:, :wlen],
                                    in1=sxw[:, :wlen], op=mybir.AluOpType.add)

            if phase == "A":
                pst = pstpool.tile([64, WIN], f32)
                nc.tensor.matmul(pst[:, :wlen], lhsT=W1_sb[:, :],
                                 rhs=zw[:, :wlen], start=True, stop=True)
                ht = htpool.tile([64, WIN], f32, tag="ht")
                nc.scalar.activation(ht[:, :wlen], pst[:, :wlen],
                                     mybir.ActivationFunctionType.Relu,
                                     bias=b1_sb[:, :])
                hp = hppool.tile([32, WIN], f32)
                nc.tensor.matmul(hp[:, :wlen], lhsT=W2_sb[:, :],
                                 rhs=ht[:, :wlen], start=True, stop=True)
                hpT = hptpool.tile([32, WIN], bf16, tag="hpT")
                nc.vector.tensor_copy(hpT[:, :wlen], hp[:, :wlen])
                nc.sync.dma_start(out=h_out[:, w * WIN:w * WIN + wlen],
                                  in_=hpT[:, :wlen])
            else:
                ht2 = htpool.tile([32, WIN], f32, tag="ht2")
                nc.scalar.activation(ht2[:, :wlen], zw[:, :wlen],
                                     mybir.ActivationFunctionType.Relu,
                                     bias=b2_sb[:, :])
                psl = plpool.tile([1, WIN], f32)
                nc.tensor.matmul(psl[:, :wlen], lhsT=lw_sb[:, :],
                                 rhs=ht2[:, :wlen], start=True, stop=True)
                otn = opool.tile([1, WIN], f32, tag="otn")
                otp = opool.tile([1, WIN], f32, tag="otp")
                nc.scalar.activation(otn[:, :wlen], psl[:, :wlen],
                                     mybir.ActivationFunctionType.Identity,
                                     bias=nlb[:, :], scale=-1.0)
                nc.scalar.activation(otp[:, :wlen], psl[:, :wlen],
                                     mybir.ActivationFunctionType.Identity,
                                     bias=lb_sb[:, :], scale=1.0)
                nc.sync.dma_start(out=outT[0:1, w * WIN:w * WIN + wlen],
                                  in_=otn[:, :wlen])
                nc.sync.dma_start(out=outT[1:2, w * WIN:w * WIN + wlen],
                                  in_=otp[:, :wlen])

    nc.compile()
    return nc


# ------------------------------------------------------------------ runner ---
def make_runner(nc, device):
    """Single-core jit runner pinned to one device, reusable across calls."""
    import jax
    import concourse.mybir as mybir
    from concourse import bass2jax

    bass2jax.install_neuronx_cc_hook()

    in_names, out_names, out_avals, zero_shapes = [], [], [], []
    for alloc in nc.m.functions[0].allocations:
        if not isinstance(alloc, mybir.MemoryLocationSet):
            continue
        nm = alloc.memorylocations[0].name
        if alloc.kind == "ExternalInput":
            in_names.append(nm)
        elif alloc.kind == "ExternalOutput":
            shape = tuple(alloc.tensor_shape)
            dtype = mybir.dt.np(alloc.dtype)
            out_names.append(nm)
            out_avals.append(jax.core.ShapedArray(shape, dtype))
            zero_shapes.append((shape, dtype))
    n_params = len(in_names)
    all_in_names = in_names + out_names
    donate = tuple(range(n_params, n_params + len(out_names)))

    def _body(*args):
        outs = bass2jax._bass_exec_p.bind(
            *args,
            out_avals=tuple(out_avals),
            in_names=tuple(all_in_names),
            out_names=tuple(out_names),
            lowering_input_output_aliases=(),
            sim_require_finite=True,
            sim_require_nnan=True,
            nc=nc,
        )
        return tuple(outs)

    jitted = jax.jit(_body, donate_argnums=donate, keep_unused=True)

    def run(in_map):
        args = [jax.device_put(np.asarray(in_map[nm]), device) for nm in in_names]
        zeros = [jax.device_put(np.zeros(s, d), device) for s, d in zero_shapes]
        outs = jitted(*args, *zeros)
        return {nm: outs[i] for i, nm in enumerate(out_names)}

    return run


# ---------------------------------------------------------------- kernel() ---
_CACHE = {}


def _get_runners(plans, cfg):
    import jax
    from concurrent.futures import ThreadPoolExecutor
    key = "runners"
    if key in _CACHE:
        return _CACHE[key]
    devices = jax.devices()[:cfg.P]

    def build_pair(d):
        ncA = build_program(plans["A"][d], "A", cfg, name=f"gnnA_d{d}")
        ncB = build_program(plans["B"][d], "B", cfg, name=f"gnnB_d{d}")
        return (make_runner(ncA, devices[d]), make_runner(ncB, devices[d]))

    with ThreadPoolExecutor(4) as exe:
        runners = list(exe.map(build_pair, range(cfg.P)))
    _CACHE[key] = runners
    return runners


def run_two_phase(inputs, cfg=FULL):
    from concurrent.futures import ThreadPoolExecutor

    x = np.asarray(inputs["x"], np.float32)
    W1 = np.asarray(inputs["W1"], np.float32)
    b1 = np.asarray(inputs["b1"], np.float32)
    W2 = np.asarray(inputs["W2"], np.float32)
    b2 = np.asarray(inputs["b2"], np.float32)
    lin_w = np.asarray(inputs["lin_w"], np.float32)
    lin_b = np.asarray(inputs["lin_b"], np.float32)

    plans, dis = preprocess(inputs["edge_index"], inputs["edge_logits"], cfg)
    dis2 = (dis * dis).astype(np.float32)
    runners = _get_runners(plans, cfg)

    x_pack = np.ascontiguousarray(x.astype(BF16).reshape(cfg.A_CELLS, 128))

    def inputs_A(d):
        sh = slice(d * cfg.NLOC, (d + 1) * cfg.NLOC)
        sxT = np.ascontiguousarray((x[sh] * dis2[sh, None]).T)
        idxall, _ = pack_idx(plans["A"][d], cfg)
        m = dict(table=x_pack, sxT=sxT, idxall=idxall,
                 W1=W1, b1c=b1.reshape(64, 1), W2p=W2)
        for q in range(4):
            m[f"S{q}"] = plans["A"][d][q]["S"]
        return m

    with ThreadPoolExecutor(cfg.P) as exe:
        resA = list(exe.map(lambda d: runners[d][0](inputs_A(d)), range(cfg.P)))
    # phase-B table: host transposes the h1p^T shards into node-order rows
    h_shards = [np.asarray(r["h_out"]) for r in resA]       # [32, NLOC] bf16
    h1p = np.concatenate([h.T for h in h_shards], axis=0)   # [N, 32]
    tableB = np.ascontiguousarray(h1p.reshape(cfg.B_CELLS, 128))

    def inputs_B(d):
        sh = slice(d * cfg.NLOC, (d + 1) * cfg.NLOC)
        sxT = np.ascontiguousarray(
            h_shards[d].astype(np.float32) * dis2[sh][None, :])
        idxall, _ = pack_idx(plans["B"][d], cfg)
        m = dict(table=tableB, sxT=sxT, idxall=idxall,
                 b2c=b2.reshape(32, 1), lw=lin_w, lb=lin_b.reshape(1, 1))
        for q in range(4):
            m[f"S{q}"] = plans["B"][d][q]["S"]
        return m

    with ThreadPoolExecutor(cfg.P) as exe:
        resB = list(exe.map(lambda d: runners[d][1](inputs_B(d)), range(cfg.P)))
    out = np.concatenate([np.asarray(r["outT"]).T for r in resB], axis=0)
    return out.astype(np.float32)


def kernel(x, edge_index, edge_logits, W1, b1, W2, b2, lin_w, lin_b):
    inputs = dict(x=x, edge_index=edge_index, edge_logits=edge_logits,
                  W1=W1, b1=b1, W2=W2, b2=b2, lin_w=lin_w, lin_b=lin_b)
    return run_two_phase(inputs, FULL)
